# revision 12
# baseline (speedup 1.0000x reference)
"""Single attention head (B=8, S=2048, D_IN=1024, D_OUT=64) on 8 TRN2 NeuronCores.

Strategy: pure data-parallel over batch -- core b computes batch element b's
full attention head. No collectives.

Per-core dataflow (v2 -- rebalanced so ScalarE's exp stream and the PE matmul
stream both run ~saturated):
  - All host-side tensors are pre-laid-out so every input DMA is contiguous
    (the v1 strided wkq/wqk descriptors stalled the first matmul ~6.5us).
    seq (fp8 and bf16) is sliced by 512-column sj chunk so compute starts
    after the first 512KB lands, not after the full 2MB/4MB.
  - DMA issue is kept off ScalarE entirely (its ~0.7us/issue was eating the
    exp budget): critical loads (wkq/wqk/misc/seqf8) go on the sync HWDGE
    queue, bulk loads (wv/seqb) on the gpsimd SWDGE queue.
  - K/Q projections in fp8 DoubleRow as in v1 (x32-scaled stacked [wk|wq] /
    [wq|wk] weights, bias via one DVE drain each, x1024 folded into the exp
    scale).  Numerics: 3.2e-3 rel err vs f32 reference (gate 2e-2).
  - scores: key-chunk pairs row-tiled on the PE (rows 0:64 / 64:128 co-run)
    into [128, 1024] psum pair tiles; ONE exp activation per pair tile.
    Schedule is qc-major: all 8 pair blocks of q-chunk 0 stream during the
    KQ projection loop, then qc 1..3 pair blocks are emitted one per "ACT
    cycle" with deferred ctx matmuls + V-projection chunks as PE filler, so
    ScalarE (the ~36.7us exp wall) never starves and the PE never idles.
  - V projection (bf16) drains to vT [64, S]; the [k, f] natural layout is
    produced by DMA-engine xbar transposes (idle DMA hardware) instead of PE
    transposes; masked-out keys' V rows and the appended ones-column are
    zeroed on DVE, applying the attention mask exactly.
  - finalize per q-chunk: ctx rows 0:64 drain to bf16 and DMA-transpose to
    [q, 64]; the denominator row (ctx row 64 = keep-mask dot exp) is
    partition-scattered by a tiny SWDGE SBUF->SBUF DMA, reciprocal'd and
    multiplied on DVE, per-q-chunk DMA out.  No PE transposes anywhere.
"""

import numpy as np
import ml_dtypes

import concourse.bass as bass  # noqa: F401  (bass types used via tile/bacc)
import concourse.mybir as mybir
import concourse.tile as tile
from concourse import bacc
from concourse.bass_utils import run_bass_kernel_spmd

B, S, D, F = 8, 2048, 1024, 64
NCORES = 8
BF = mybir.dt.bfloat16
F8 = mybir.dt.float8e4
F32 = mybir.dt.float32
# reference scales by sqrt(S); q and k each carry x32 from the fp8 weight scaling
SCALE = 1.0 / (1024.0 * float(np.sqrt(np.float32(S))))
SC = 512  # matmul moving free-dim
NSJ = S // SC  # 4 column chunks of the projection loop
KCH = S // 128  # 16 key chunks
DCH = D // 128  # 8 bf16 contraction chunks
DR = D // 256  # 4 fp8 DoubleRow contraction chunks
NDUMMY = 4  # HAM warmup matmuls while the first seq chunk is in flight


def _emit(nc):
    # all layouts pre-arranged host-side so every DMA below is contiguous
    seqf8_d = nc.declare_dram_parameter("seqf8", [NSJ, 128, DR, 2, SC], F8, isOutput=False)
    seqb_d = nc.declare_dram_parameter("seqb", [NSJ, 128, DCH, SC], BF, isOutput=False)
    wkq_d = nc.declare_dram_parameter("wkq", [128, DR, 2, 128], F8, isOutput=False)
    wqk_d = nc.declare_dram_parameter("wqk", [128, DR, 2, 128], F8, isOutput=False)
    wv_d = nc.declare_dram_parameter("wv", [128, DCH, F], BF, isOutput=False)
    # misc f32 [128, 19]: col0 = 32*[bk; bq] stacked, col1 = 32*[bq; bk],
    # col2 rows0:64 = bv, cols 3:19 = 0/1 keep-mask per key chunk [128, 16]
    misc_d = nc.declare_dram_parameter("misc", [128, 3 + KCH], F32, isOutput=False)
    out_d = nc.declare_dram_parameter("out", [S, F], F32, isOutput=True)

    with tile.TileContext(nc) as tc:
        _body(nc, tc, seqf8_d, seqb_d, wkq_d, wqk_d, wv_d, misc_d, out_d)
    nc.compile()


def _body(nc, tc, seqf8_d, seqb_d, wkq_d, wqk_d, wv_d, misc_d, out_d):
    from contextlib import ExitStack

    with ExitStack() as ctx:
        const = ctx.enter_context(tc.tile_pool(name="const", bufs=1))
        big = ctx.enter_context(tc.tile_pool(name="big", bufs=1))
        sbw = ctx.enter_context(tc.tile_pool(name="sbw", bufs=1))
        ps = ctx.enter_context(tc.tile_pool(name="ps", space="PSUM", bufs=1))

        # ---- input DMAs: critical path on the sync HWDGE queue ----
        wkq_sb = const.tile([128, DR, 2, 128], F8, name="wkq_sb")
        nc.sync.dma_start(out=wkq_sb[:], in_=wkq_d.ap())
        wqk_sb = const.tile([128, DR, 2, 128], F8, name="wqk_sb")
        nc.sync.dma_start(out=wqk_sb[:], in_=wqk_d.ap())
        misc_sb = const.tile([128, 3 + KCH], F32, name="misc_sb")
        nc.sync.dma_start(out=misc_sb[:], in_=misc_d.ap())
        seqf8 = []
        for j in range(NSJ):
            t = big.tile([128, DR, 2, SC], F8, name=f"seqf8_{j}")
            nc.sync.dma_start(out=t[:], in_=seqf8_d[j])
            seqf8.append(t)
        # ---- bulk loads on the gpsimd SWDGE queue ----
        wv_sb = const.tile([128, DCH, F], BF, name="wv_sb")
        nc.gpsimd.dma_start(out=wv_sb[:], in_=wv_d.ap())
        seqb = []
        for j in range(NSJ):
            t = big.tile([128, DCH, SC], BF, name=f"seqb_{j}")
            nc.gpsimd.dma_start(out=t[:], in_=seqb_d[j])
            seqb.append(t)

        # preload the exp table set so the ~2.7us table DMA overlaps the loads
        dummy_sb = const.tile([1, 1], F32, name="dummy_sb")
        nc.scalar.activation(
            out=dummy_sb[:],
            in_=misc_sb[0:1, 0:1],
            func=mybir.ActivationFunctionType.Exp,
            scale=1.0,
        )

        # kqT: k on rows 0:64 (pair A lhsT), q on rows 64:128 (pair B rhs)
        # kq2T (reversed stacking): q on rows 0:64 (pair A rhs), k on rows
        # 64:128 (pair B lhsT)
        kqT_sb = big.tile([128, S], BF, name="kqT_sb")
        kq2T_sb = big.tile([128, S], BF, name="kq2T_sb")
        vT_sb = big.tile([F, S], BF, name="vT_sb")
        vt_stage = big.tile([128, KCH, F], BF, name="vt_stage")
        v_sb = big.tile([128, KCH, F + 1], BF, name="v_sb")
        recT = big.tile([128, KCH], F32, name="recT")
        out_sb = big.tile([128, KCH, F], F32, name="out_sb")
        out_r = out_d.ap().rearrange("(c p) f -> p c f", p=128)

        bkq_ap = misc_sb[:, 0:1]  # stacked 32*[bk; bq]
        bqk_ap = misc_sb[:, 1:2]  # stacked 32*[bq; bk]
        bv_ap = misc_sb[0:F, 2:3]
        mask01 = misc_sb[:, 3:]  # [128, 16] 1.0 = keep, 0.0 = masked out

        # ones-column of v := keep-mask (masked keys contribute 0 to the sums)
        nc.vector.tensor_copy(v_sb[:, :, F], mask01)

        # ---- HAM warmup: a few junk matmuls while the first seq chunk is in
        # flight, so the PE clock-gate is at 8/8 when real work starts ----
        wkq_flat = wkq_sb.rearrange("p a b c -> p (a b c)")
        for i in range(NDUMMY):
            ps_warm = ps.tile([128, SC], F32, tag="pk", bufs=2, name=f"ps_warm{i}")
            nc.tensor.matmul(
                ps_warm[:], wkq_flat[:, 0:128], wkq_flat[:, 0:SC],
                start=True, stop=True,
            )

        ctx_tiles = {}
        pending_ctx = []  # deferred ctx matmuls -- popped as PE filler

        def emit_ctx(qc, p, expq):
            ctx_ps = ctx_tiles[qc]
            ka, kb = 2 * p, 2 * p + 1
            nc.tensor.matmul(
                ctx_ps[:], v_sb[:, ka, :], expq[:, 0:SC], start=(p == 0), stop=False
            )
            nc.tensor.matmul(
                ctx_ps[:],
                v_sb[:, kb, :],
                expq[:, SC : 2 * SC],
                start=False,
                stop=(p == KCH // 2 - 1),
            )

        def pop_ctx(n):
            for _ in range(min(n, len(pending_ctx))):
                qc, p, expq = pending_ctx.pop(0)
                emit_ctx(qc, p, expq)
                if p == KCH // 2 - 1:
                    # that was qc's last ctx matmul -- drain it now so its
                    # psum slot frees up and the output DMA overlaps
                    finalize(qc)

        def pair_block(qc, p):
            # scores for key chunks (2p, 2p+1) x q-chunk qc, then exp.
            qsl = slice(qc * SC, (qc + 1) * SC)
            if qc not in ctx_tiles:
                ctx_tiles[qc] = ps.tile(
                    [F + 1, SC], F32, tag="ctx", bufs=2, name=f"ctx_ps{qc}"
                )
            ka, kb = 2 * p, 2 * p + 1
            ps_pair = ps.tile(
                [128, 2 * SC], F32, tag="pair", bufs=2, name=f"ps_pair_{qc}_{p}"
            )
            # chunk A on array rows 0:64, chunk B on rows 64:128 --
            # disjoint row groups run concurrently on the PE
            nc.tensor.matmul(
                ps_pair[:, 0:SC],
                kqT_sb[0:F, ka * 128 : (ka + 1) * 128],
                kq2T_sb[0:F, qsl],
                start=True,
                stop=True,
            )
            nc.tensor.matmul(
                ps_pair[:, SC : 2 * SC],
                kq2T_sb[64:128, kb * 128 : (kb + 1) * 128],
                kqT_sb[64:128, qsl],
                start=True,
                stop=True,
            )
            expq = sbw.tile(
                [128, 2 * SC], BF, tag="expq", bufs=12, name=f"expq_{qc}_{p}"
            )
            nc.scalar.activation(
                out=expq[:],
                in_=ps_pair[:],
                func=mybir.ActivationFunctionType.Exp,
                scale=SCALE,
            )
            pending_ctx.append((qc, p, expq))

        # ---- V-projection filler units (PE work to fill exp-paced slack) ----
        vps = {}

        def v_unit(u):
            sj, h = divmod(u, 2)
            if h == 0:
                vps[sj] = ps.tile([F, SC], F32, tag="pk", bufs=2, name=f"ps_v{sj}")
            for c in range(4 * h, 4 * h + 4):
                nc.tensor.matmul(
                    vps[sj][:],
                    wv_sb[:, c, :],
                    seqb[sj][:, c, :],
                    start=(c == 0),
                    stop=(c == DCH - 1),
                )
            if h == 1:
                sl = slice(sj * SC, (sj + 1) * SC)
                nc.vector.tensor_scalar_add(vT_sb[:, sl], vps[sj][:], bv_ap)
                # xbar transposes [64, 128] -> [128, 64] per key chunk
                for t in range(4 * sj, 4 * sj + 4):
                    nc.sync.dma_start_transpose(
                        out=vt_stage[:, t, :],
                        in_=vT_sb[:, t * 128 : (t + 1) * 128],
                    )
                    nc.vector.tensor_scalar_mul(
                        v_sb[:, t, 0:F], vt_stage[:, t, :], mask01[:, t : t + 1]
                    )

        def finalize(qc):
            # ctx [65, 512] -> bf16, then xbar-transpose [80, 128] chunks
            # (80 = next multiple of 16) so transposed col 64 IS the softmax
            # denominator; reciprocal + row scale on DVE; per-q-chunk DMA out
            ctx_ps = ctx_tiles.pop(qc)
            ctxb = sbw.tile([80, SC], BF, tag="ctxb", bufs=2, name=f"ctxb{qc}")
            if qc < 2:  # first use of each ring slot: define the pad rows
                # (16-partition-aligned; the copy below rewrites row 64)
                nc.gpsimd.memset(ctxb[F : 80, :], 0.0)
            nc.vector.tensor_copy(ctxb[0 : F + 1, :], ctx_ps[:])
            ctq = sbw.tile([128, 4, 80], BF, tag="ctq", bufs=2, name=f"ctq{qc}")
            for i in range(4):
                nc.sync.dma_start_transpose(
                    out=ctq[:, i, :], in_=ctxb[:, i * 128 : (i + 1) * 128]
                )
            nc.vector.reciprocal(
                recT[:, 4 * qc : 4 * qc + 4], ctq[:, :, F : F + 1].rearrange("p t o -> p (t o)")
            )
            for i in range(4):
                t = qc * 4 + i
                nc.vector.tensor_scalar_mul(
                    out_sb[:, t, :], ctq[:, i, 0:F], recT[:, t : t + 1]
                )
            nc.sync.dma_start(
                out=out_r[:, qc * 4 : (qc + 1) * 4, :],
                in_=out_sb[:, qc * 4 : (qc + 1) * 4, :],
            )

        # ---- Phase A: K/Q projections with q-chunk 0's pair blocks ----
        for sj in range(NSJ):
            ps_kq = ps.tile([128, SC], F32, tag="pk", bufs=2, name=f"ps_kq{sj}")
            ps_kq2 = ps.tile([128, SC], F32, tag="pk", bufs=2, name=f"ps_kq2_{sj}")
            for c in range(DR):
                rhs = seqf8[sj][:, c, :, :]
                st = dict(start=(c == 0), stop=(c == DR - 1))
                nc.tensor.matmul(
                    ps_kq[:], wkq_sb[:, c, :, :], rhs,
                    perf_mode=mybir.MatmulPerfMode.DoubleRow, **st
                )
                nc.tensor.matmul(
                    ps_kq2[:], wqk_sb[:, c, :, :], rhs,
                    perf_mode=mybir.MatmulPerfMode.DoubleRow, **st
                )
            sl = slice(sj * SC, (sj + 1) * SC)
            nc.vector.tensor_scalar_add(kqT_sb[:, sl], ps_kq[:], bkq_ap)
            nc.vector.tensor_scalar_add(kq2T_sb[:, sl], ps_kq2[:], bqk_ap)
            pair_block(0, 2 * sj)
            pair_block(0, 2 * sj + 1)

        # ---- Phases B/C/D: qc 1..3 pair blocks, exp-paced; filler = V
        # projection units (phase B) and deferred ctx pops, distributed so
        # the PE tracks the ~1.15us/block exp pace ----
        for p in range(KCH // 2):  # qc = 1: all V units + a few pops
            pair_block(1, p)
            v_unit(p)
            if p % 2 == 1:
                pop_ctx(1)
        for p in range(KCH // 2):  # qc = 2: catch up on pops
            pair_block(2, p)
            pop_ctx(2 if p % 2 == 1 else 1)
        for p in range(KCH // 2):  # qc = 3: two pops per block retires all
            pair_block(3, p)
            pop_ctx(2)
        pop_ctx(len(pending_ctx))


_NC_CACHE = None


def _get_nc():
    global _NC_CACHE
    if _NC_CACHE is None:
        nc = bacc.Bacc("TRN2", target_bir_lowering=False, debug=False)
        _emit(nc)
        _NC_CACHE = nc
    return _NC_CACHE


def make_in_maps(seq, mask, Wq, bq, Wk, bk, Wv, bv):
    bf16 = ml_dtypes.bfloat16
    f8 = ml_dtypes.float8_e4m3
    seq = np.asarray(seq, dtype=np.float32)
    mask = np.asarray(mask).astype(bool)
    wkq = np.concatenate(
        [np.asarray(Wk, dtype=np.float32), np.asarray(Wq, dtype=np.float32)], axis=1
    )  # [D, 128]
    wqk = np.concatenate(
        [np.asarray(Wq, dtype=np.float32), np.asarray(Wk, dtype=np.float32)], axis=1
    )
    # DoubleRow layout [p, c, i, f] for row index d = 256c + 2p + i, contiguous
    wkq_h = np.ascontiguousarray(
        (wkq * 32.0).astype(f8).reshape(DR, 128, 2, 128).transpose(1, 0, 2, 3)
    )
    wqk_h = np.ascontiguousarray(
        (wqk * 32.0).astype(f8).reshape(DR, 128, 2, 128).transpose(1, 0, 2, 3)
    )
    wv_h = np.ascontiguousarray(
        np.asarray(Wv, dtype=np.float32).astype(bf16).reshape(DCH, 128, F).transpose(1, 0, 2)
    )
    in_maps = []
    for b in range(NCORES):
        seqT = np.ascontiguousarray(seq[b].T)  # [D, S] f32
        # fp8, sliced by sj: [sj, p, c, i, t]
        sf8 = np.ascontiguousarray(
            seqT.astype(f8).reshape(DR, 128, 2, NSJ, SC).transpose(3, 1, 0, 2, 4)
        )
        # bf16, sliced by sj: [sj, p, c, t]
        sb16 = np.ascontiguousarray(
            seqT.astype(bf16).reshape(DCH, 128, NSJ, SC).transpose(2, 1, 0, 3)
        )
        misc = np.zeros((128, 3 + KCH), dtype=np.float32)
        misc[0:F, 0] = 32.0 * np.asarray(bk, dtype=np.float32)
        misc[64:128, 0] = 32.0 * np.asarray(bq, dtype=np.float32)
        misc[0:F, 1] = 32.0 * np.asarray(bq, dtype=np.float32)
        misc[64:128, 1] = 32.0 * np.asarray(bk, dtype=np.float32)
        misc[0:F, 2] = np.asarray(bv, dtype=np.float32)
        # keep-mask: misc[p, 3+c] = 0.0 if key c*128+p is masked out else 1.0
        misc[:, 3:] = np.where(mask[b], np.float32(0.0), np.float32(1.0)).reshape(
            KCH, 128
        ).T
        in_maps.append(
            {
                "seqf8": sf8,
                "seqb": sb16,
                "wkq": wkq_h,
                "wqk": wqk_h,
                "wv": wv_h,
                "misc": misc,
            }
        )
    return in_maps


def run(in_maps, trace=False, **kw):
    nc = _get_nc()
    return run_bass_kernel_spmd(
        nc, in_maps, core_ids=list(range(NCORES)), trace=trace, **kw
    )


def kernel(seq, mask, Wq, bq, Wk, bk, Wv, bv):
    in_maps = make_in_maps(seq, mask, Wq, bq, Wk, bk, Wv, bv)
    res = run(in_maps)
    out = np.stack(
        [np.asarray(res.results[i]["out"], dtype=np.float32) for i in range(NCORES)],
        axis=0,
    )
    return out


# revision 19
# speedup vs baseline: 1.4589x; 1.4589x over previous
"""Single attention head (B=8, S=2048, D_IN=1024, D_OUT=64) on 8 TRN2 NeuronCores.

Strategy: pure data-parallel over batch -- core b computes batch element b's
full attention head. No collectives.

Per-core dataflow (v2 -- rebalanced so ScalarE's exp stream and the PE matmul
stream both run ~saturated):
  - All host-side tensors are pre-laid-out so every input DMA is contiguous
    (the v1 strided wkq/wqk descriptors stalled the first matmul ~6.5us).
    seq (fp8 and bf16) is sliced by 512-column sj chunk so compute starts
    after the first 512KB lands, not after the full 2MB/4MB.
  - ALL DMA issue lives on the sync HWDGE queue in priority order (weights,
    first fp8 seq chunk, misc, remaining fp8, identities, bf16 seq); ScalarE
    issues nothing so its exp stream is unimpeded.  (SWDGE/gpsimd DMAs cost
    ~3.8us of engine DRAIN each; DMA-engine xbar transposes cost ~1.2us of
    issue each and head-of-line block the sync FIFO -- both measured, both
    avoided.)
  - K/Q projections in fp8 DoubleRow as in v1 (x32-scaled stacked [wk|wq] /
    [wq|wk] weights, bias via one DVE drain each, x1024 folded into the exp
    scale).  Numerics: 3.2e-3 rel err vs f32 reference (gate 2e-2).
  - scores: key-chunk pairs row-tiled on the PE (rows 0:64 / 64:128 co-run)
    into [128, 1024] psum pair tiles; ONE exp activation per pair tile.
    Schedule is qc-major: all 8 pair blocks of q-chunk 0 stream during the
    KQ projection loop, then qc 1..3 pair blocks are emitted one per "ACT
    cycle" with deferred ctx matmuls + V-projection chunks as PE filler, so
    ScalarE (the ~36.7us exp wall) never starves and the PE never idles.
  - V projection (bf16) drains to vT [64, S]; vT re-transposed on the PE
    into natural [k, f] layout; masked-out keys' V rows and the appended
    ones-column are zeroed on DVE, applying the attention mask exactly.
  - finalize per q-chunk (as soon as its last ctx matmul retires): drain
    ctx, PE-transpose back to [q, 65], multiply rows by 1/ctx[.., 64],
    per-q-chunk DMA out overlapping the remaining compute.
"""

import numpy as np
import ml_dtypes

import concourse.bass as bass  # noqa: F401  (bass types used via tile/bacc)
import concourse.mybir as mybir
import concourse.tile as tile
from concourse import bacc
from concourse.bass_utils import run_bass_kernel_spmd

B, S, D, F = 8, 2048, 1024, 64
NCORES = 8
BF = mybir.dt.bfloat16
F8 = mybir.dt.float8e4
F32 = mybir.dt.float32
# reference scales by sqrt(S); q and k each carry x32 from the fp8 weight scaling
SCALE = 1.0 / (1024.0 * float(np.sqrt(np.float32(S))))
SC = 512  # matmul moving free-dim
NSJ = S // SC  # 4 column chunks of the projection loop
KCH = S // 128  # 16 key chunks
DCH = D // 128  # 8 bf16 contraction chunks
DR = D // 256  # 4 fp8 DoubleRow contraction chunks
NDUMMY = 4  # HAM warmup matmuls while the first seq chunk is in flight


def _emit(nc):
    # all layouts pre-arranged host-side so every DMA below is contiguous
    seqf8_d = nc.declare_dram_parameter("seqf8", [NSJ, 128, DR, 2, SC], F8, isOutput=False)
    seqb_d = nc.declare_dram_parameter("seqb", [NSJ, 128, DCH, SC], BF, isOutput=False)
    wkq_d = nc.declare_dram_parameter("wkq", [128, DR, 2, 128], F8, isOutput=False)
    wqk_d = nc.declare_dram_parameter("wqk", [128, DR, 2, 128], F8, isOutput=False)
    wv_d = nc.declare_dram_parameter("wv", [128, DCH, F], BF, isOutput=False)
    # misc f32 [128, 19]: col0 = 32*[bk; bq] stacked, col1 = 32*[bq; bk],
    # col2 rows0:64 = bv, cols 3:19 = 0/1 keep-mask per key chunk [128, 16]
    misc_d = nc.declare_dram_parameter("misc", [128, 3 + KCH], F32, isOutput=False)
    identb_d = nc.declare_dram_parameter("identb", [128, 128], BF, isOutput=False)
    identf_d = nc.declare_dram_parameter("identf", [128, 128], F32, isOutput=False)
    out_d = nc.declare_dram_parameter("out", [S, F], F32, isOutput=True)

    with tile.TileContext(nc) as tc:
        _body(nc, tc, seqf8_d, seqb_d, wkq_d, wqk_d, wv_d, misc_d, identb_d, identf_d, out_d)
    nc.compile()


def _body(nc, tc, seqf8_d, seqb_d, wkq_d, wqk_d, wv_d, misc_d, identb_d, identf_d, out_d):
    from contextlib import ExitStack

    with ExitStack() as ctx:
        const = ctx.enter_context(tc.tile_pool(name="const", bufs=1))
        big = ctx.enter_context(tc.tile_pool(name="big", bufs=1))
        sbw = ctx.enter_context(tc.tile_pool(name="sbw", bufs=1))
        ps = ctx.enter_context(tc.tile_pool(name="ps", space="PSUM", bufs=1))

        # ---- input DMAs: all on the sync HWDGE queue (FIFO = priority
        # order); ScalarE issues nothing so its exp stream is unimpeded ----
        wkq_sb = const.tile([128, DR, 2, 128], F8, name="wkq_sb")
        nc.sync.dma_start(out=wkq_sb[:], in_=wkq_d.ap())
        wqk_sb = const.tile([128, DR, 2, 128], F8, name="wqk_sb")
        nc.sync.dma_start(out=wqk_sb[:], in_=wqk_d.ap())
        seqf8 = []
        for j in range(NSJ):
            t = big.tile([128, DR, 2, SC], F8, name=f"seqf8_{j}")
            seqf8.append(t)
        nc.sync.dma_start(out=seqf8[0][:], in_=seqf8_d[0])
        misc_sb = const.tile([128, 3 + KCH], F32, name="misc_sb")
        nc.sync.dma_start(out=misc_sb[:], in_=misc_d.ap())
        for j in range(1, NSJ):
            nc.sync.dma_start(out=seqf8[j][:], in_=seqf8_d[j])
        identb_sb = const.tile([128, 128], BF, name="identb_sb")
        nc.sync.dma_start(out=identb_sb[:], in_=identb_d.ap())
        identf_sb = const.tile([128, 128], F32, name="identf_sb")
        nc.sync.dma_start(out=identf_sb[:], in_=identf_d.ap())
        wv_sb = const.tile([128, DCH, F], BF, name="wv_sb")
        nc.sync.dma_start(out=wv_sb[:], in_=wv_d.ap())
        seqb = []
        for j in range(NSJ):
            t = big.tile([128, DCH, SC], BF, name=f"seqb_{j}")
            nc.sync.dma_start(out=t[:], in_=seqb_d[j])
            seqb.append(t)

        # preload the exp table set so the ~2.7us table DMA overlaps the loads
        dummy_sb = const.tile([1, 1], F32, name="dummy_sb")
        nc.scalar.activation(
            out=dummy_sb[:],
            in_=misc_sb[0:1, 0:1],
            func=mybir.ActivationFunctionType.Exp,
            scale=1.0,
        )

        # kqT: k on rows 0:64 (pair A lhsT), q on rows 64:128 (pair B rhs)
        # kq2T (reversed stacking): q on rows 0:64 (pair A rhs), k on rows
        # 64:128 (pair B lhsT)
        kqT_sb = big.tile([128, S], BF, name="kqT_sb")
        kq2T_sb = big.tile([128, S], BF, name="kq2T_sb")
        vT_sb = big.tile([F, S], BF, name="vT_sb")
        v_sb = big.tile([128, KCH, F + 1], BF, name="v_sb")
        recT = big.tile([128, KCH], F32, name="recT")
        out_sb = big.tile([128, KCH, F], F32, name="out_sb")
        out_r = out_d.ap().rearrange("(c p) f -> p c f", p=128)

        bkq_ap = misc_sb[:, 0:1]  # stacked 32*[bk; bq]
        bqk_ap = misc_sb[:, 1:2]  # stacked 32*[bq; bk]
        bv_ap = misc_sb[0:F, 2:3]
        mask01 = misc_sb[:, 3:]  # [128, 16] 1.0 = keep, 0.0 = masked out

        # ones-column of v := keep-mask (masked keys contribute 0 to the sums)
        nc.vector.tensor_copy(v_sb[:, :, F], mask01)

        # ---- HAM warmup: a few junk matmuls while the first seq chunk is in
        # flight, so the PE clock-gate is at 8/8 when real work starts ----
        wkq_flat = wkq_sb.rearrange("p a b c -> p (a b c)")
        for i in range(NDUMMY):
            ps_warm = ps.tile([128, SC], F32, tag="pk", bufs=2, name=f"ps_warm{i}")
            nc.tensor.matmul(
                ps_warm[:], wkq_flat[:, 0:128], wkq_flat[:, 0:SC],
                start=True, stop=True,
            )

        ctx_tiles = {}
        pending_ctx = []  # deferred ctx matmuls -- popped as PE filler

        def emit_ctx(qc, p, expq):
            ctx_ps = ctx_tiles[qc]
            ka, kb = 2 * p, 2 * p + 1
            nc.tensor.matmul(
                ctx_ps[:], v_sb[:, ka, :], expq[:, 0:SC], start=(p == 0), stop=False
            )
            nc.tensor.matmul(
                ctx_ps[:],
                v_sb[:, kb, :],
                expq[:, SC : 2 * SC],
                start=False,
                stop=(p == KCH // 2 - 1),
            )

        def pop_ctx(n):
            for _ in range(min(n, len(pending_ctx))):
                qc, p, expq = pending_ctx.pop(0)
                emit_ctx(qc, p, expq)
                if p == KCH // 2 - 1:
                    # that was qc's last ctx matmul -- drain it now so its
                    # psum slot frees up and the output DMA overlaps
                    finalize(qc)

        def pair_block(qc, p):
            # scores for key chunks (2p, 2p+1) x q-chunk qc, then exp.
            qsl = slice(qc * SC, (qc + 1) * SC)
            if qc not in ctx_tiles:
                ctx_tiles[qc] = ps.tile(
                    [F + 1, SC], F32, tag="ctx", bufs=2, name=f"ctx_ps{qc}"
                )
            ka, kb = 2 * p, 2 * p + 1
            ps_pair = ps.tile(
                [128, 2 * SC], F32, tag="pair", bufs=2, name=f"ps_pair_{qc}_{p}"
            )
            # chunk A on array rows 0:64, chunk B on rows 64:128 --
            # disjoint row groups run concurrently on the PE
            nc.tensor.matmul(
                ps_pair[:, 0:SC],
                kqT_sb[0:F, ka * 128 : (ka + 1) * 128],
                kq2T_sb[0:F, qsl],
                start=True,
                stop=True,
            )
            nc.tensor.matmul(
                ps_pair[:, SC : 2 * SC],
                kq2T_sb[64:128, kb * 128 : (kb + 1) * 128],
                kqT_sb[64:128, qsl],
                start=True,
                stop=True,
            )
            expq = sbw.tile(
                [128, 2 * SC], BF, tag="expq", bufs=12, name=f"expq_{qc}_{p}"
            )
            nc.scalar.activation(
                out=expq[:],
                in_=ps_pair[:],
                func=mybir.ActivationFunctionType.Exp,
                scale=SCALE,
            )
            pending_ctx.append((qc, p, expq))

        # ---- V-projection filler units (PE work to fill exp-paced slack) ----
        vps = {}

        def v_unit(u):
            sj, h = divmod(u, 2)
            if h == 0:
                vps[sj] = ps.tile([F, SC], F32, tag="pk", bufs=2, name=f"ps_v{sj}")
            for c in range(4 * h, 4 * h + 4):
                nc.tensor.matmul(
                    vps[sj][:],
                    wv_sb[:, c, :],
                    seqb[sj][:, c, :],
                    start=(c == 0),
                    stop=(c == DCH - 1),
                )
            if h == 1:
                sl = slice(sj * SC, (sj + 1) * SC)
                nc.vector.tensor_scalar_add(vT_sb[:, sl], vps[sj][:], bv_ap)
                # transpose into natural [k, f] layout on the PE
                for t in range(4 * sj, 4 * sj + 4):
                    vtp = ps.tile([128, F], BF, tag="pk", bufs=2, name=f"vtp{t}")
                    nc.tensor.transpose(
                        vtp[:],
                        vT_sb[:, t * 128 : (t + 1) * 128],
                        identb_sb[0:F, 0:F],
                    )
                    nc.vector.tensor_scalar_mul(
                        v_sb[:, t, 0:F], vtp[:], mask01[:, t : t + 1]
                    )

        def finalize(qc):
            # drain ctx, transpose back to [q, 65] on the PE, normalize, store
            ctx_ps = ctx_tiles.pop(qc)
            ctxTq = sbw.tile([F + 1, SC], F32, tag="ctxTq", bufs=2, name=f"ctxTq{qc}")
            nc.vector.tensor_copy(ctxTq[:], ctx_ps[:])
            for i in range(SC // 128):
                t = qc * 4 + i
                ctp = ps.tile([128, F + 1], F32, tag="pk", bufs=2, name=f"ctp{t}")
                nc.tensor.transpose(
                    ctp[:],
                    ctxTq[:, i * 128 : (i + 1) * 128],
                    identf_sb[0 : F + 1, 0 : F + 1],
                )
                nc.vector.reciprocal(recT[:, t : t + 1], ctp[:, F : F + 1])
                nc.vector.tensor_scalar_mul(
                    out_sb[:, t, :], ctp[:, 0:F], recT[:, t : t + 1]
                )
            nc.sync.dma_start(
                out=out_r[:, qc * 4 : (qc + 1) * 4, :],
                in_=out_sb[:, qc * 4 : (qc + 1) * 4, :],
            )

        # ---- Phase A: K/Q projections with q-chunk 0's pair blocks ----
        for sj in range(NSJ):
            ps_kq = ps.tile([128, SC], F32, tag="pk", bufs=2, name=f"ps_kq{sj}")
            ps_kq2 = ps.tile([128, SC], F32, tag="pk", bufs=2, name=f"ps_kq2_{sj}")
            for c in range(DR):
                rhs = seqf8[sj][:, c, :, :]
                st = dict(start=(c == 0), stop=(c == DR - 1))
                nc.tensor.matmul(
                    ps_kq[:], wkq_sb[:, c, :, :], rhs,
                    perf_mode=mybir.MatmulPerfMode.DoubleRow, **st
                )
                nc.tensor.matmul(
                    ps_kq2[:], wqk_sb[:, c, :, :], rhs,
                    perf_mode=mybir.MatmulPerfMode.DoubleRow, **st
                )
            sl = slice(sj * SC, (sj + 1) * SC)
            nc.vector.tensor_scalar_add(kqT_sb[:, sl], ps_kq[:], bkq_ap)
            nc.vector.tensor_scalar_add(kq2T_sb[:, sl], ps_kq2[:], bqk_ap)
            pair_block(0, 2 * sj)
            pair_block(0, 2 * sj + 1)

        # ---- Phases B/C/D: qc 1..3 pair blocks, exp-paced; filler = V
        # projection units (phase B) and deferred ctx pops, distributed so
        # the PE tracks the ~1.15us/block exp pace ----
        for p in range(KCH // 2):  # qc = 1: all V units + a few pops
            pair_block(1, p)
            v_unit(p)
            if p % 2 == 1:
                pop_ctx(1)
        for p in range(KCH // 2):  # qc = 2: catch up on pops
            pair_block(2, p)
            pop_ctx(2 if p % 2 == 1 else 1)
        for p in range(KCH // 2):  # qc = 3: two pops per block retires all
            pair_block(3, p)
            pop_ctx(2)
        pop_ctx(len(pending_ctx))


_NC_CACHE = None


def _get_nc():
    global _NC_CACHE
    if _NC_CACHE is None:
        nc = bacc.Bacc("TRN2", target_bir_lowering=False, debug=False)
        _emit(nc)
        _NC_CACHE = nc
    return _NC_CACHE


def make_in_maps(seq, mask, Wq, bq, Wk, bk, Wv, bv):
    bf16 = ml_dtypes.bfloat16
    f8 = ml_dtypes.float8_e4m3
    seq = np.asarray(seq, dtype=np.float32)
    mask = np.asarray(mask).astype(bool)
    wkq = np.concatenate(
        [np.asarray(Wk, dtype=np.float32), np.asarray(Wq, dtype=np.float32)], axis=1
    )  # [D, 128]
    wqk = np.concatenate(
        [np.asarray(Wq, dtype=np.float32), np.asarray(Wk, dtype=np.float32)], axis=1
    )
    # DoubleRow layout [p, c, i, f] for row index d = 256c + 2p + i, contiguous
    wkq_h = np.ascontiguousarray(
        (wkq * 32.0).astype(f8).reshape(DR, 128, 2, 128).transpose(1, 0, 2, 3)
    )
    wqk_h = np.ascontiguousarray(
        (wqk * 32.0).astype(f8).reshape(DR, 128, 2, 128).transpose(1, 0, 2, 3)
    )
    wv_h = np.ascontiguousarray(
        np.asarray(Wv, dtype=np.float32).astype(bf16).reshape(DCH, 128, F).transpose(1, 0, 2)
    )
    identb = np.eye(128, dtype=bf16)
    identf = np.eye(128, dtype=np.float32)
    in_maps = []
    for b in range(NCORES):
        seqT = np.ascontiguousarray(seq[b].T)  # [D, S] f32
        # fp8, sliced by sj: [sj, p, c, i, t]
        sf8 = np.ascontiguousarray(
            seqT.astype(f8).reshape(DR, 128, 2, NSJ, SC).transpose(3, 1, 0, 2, 4)
        )
        # bf16, sliced by sj: [sj, p, c, t]
        sb16 = np.ascontiguousarray(
            seqT.astype(bf16).reshape(DCH, 128, NSJ, SC).transpose(2, 1, 0, 3)
        )
        misc = np.zeros((128, 3 + KCH), dtype=np.float32)
        misc[0:F, 0] = 32.0 * np.asarray(bk, dtype=np.float32)
        misc[64:128, 0] = 32.0 * np.asarray(bq, dtype=np.float32)
        misc[0:F, 1] = 32.0 * np.asarray(bq, dtype=np.float32)
        misc[64:128, 1] = 32.0 * np.asarray(bk, dtype=np.float32)
        misc[0:F, 2] = np.asarray(bv, dtype=np.float32)
        # keep-mask: misc[p, 3+c] = 0.0 if key c*128+p is masked out else 1.0
        misc[:, 3:] = np.where(mask[b], np.float32(0.0), np.float32(1.0)).reshape(
            KCH, 128
        ).T
        in_maps.append(
            {
                "seqf8": sf8,
                "seqb": sb16,
                "wkq": wkq_h,
                "wqk": wqk_h,
                "wv": wv_h,
                "misc": misc,
                "identb": identb,
                "identf": identf,
            }
        )
    return in_maps


def run(in_maps, trace=False, **kw):
    nc = _get_nc()
    return run_bass_kernel_spmd(
        nc, in_maps, core_ids=list(range(NCORES)), trace=trace, **kw
    )


def kernel(seq, mask, Wq, bq, Wk, bk, Wv, bv):
    in_maps = make_in_maps(seq, mask, Wq, bq, Wk, bk, Wv, bv)
    res = run(in_maps)
    out = np.stack(
        [np.asarray(res.results[i]["out"], dtype=np.float32) for i in range(NCORES)],
        axis=0,
    )
    return out
